# revision 4
# baseline (speedup 1.0000x reference)
"""DeepseekV2 MoE layer on 8 Trainium2 NeuronCores (Bass/Tile, SPMD).

Strategy (expert-parallel with intermediate-dim pair-split, bf16 matmuls):
 - Host computes the MoE gate routing in numpy (bitwise-matches the jax
   reference: top-k margins are ~1e-4, far above ulp noise).
 - 16 experts, rank-sorted by token count, are dealt into 4 groups of 4
   (group p = ranks {p, 4+p, 8+p, 12+p}).  Group p is owned by the core
   pair (2p, 2p+1): both cores process the SAME gathered tokens of all 4
   experts, but each core only computes HALF of every expert's
   intermediate dim (704 rows, zero-padded to 768 = 6 tile-pairs).  The
   two half outputs are partial sums; the host adds them.  This beats
   whole-expert placement because per-slot capacity is the max count at
   ranks {0,4,8,12} (688+432+352+272) instead of ranks {0,8} (688+352)
   at double width.
 - Device per core: for each of 4 slots, GEMM1 (x_gathered^T @ half
   w_gate_up^T, gate/up rows interleaved in 128-row pairs) -> SiLU*mul
   -> GEMM2 (half w_down) -> scale rows by renormalized gate weight *
   2.5 -> DMA out.  Then the shared expert, TP-sharded over its
   intermediate dim (352 per core, zero-padded to 384).
 - Phases are emitted interleaved (GEMM1 of slot s+1 before GEMM2 of
   slot s) so the PE never idles on a slot's silu/mul tail; slots run
   smallest-first so the startup x-DMA chase is short.
 - All matmuls run in bf16 (fp32 PSUM accumulate): halves HBM traffic
   and enables fast-weight-load; rel err ~5e-3 vs the 2e-2 gate.
 - Host scatter-adds per-expert partial outputs and shared partials, f32.
"""

import numpy as np
import ml_dtypes
from contextlib import ExitStack

import concourse.bacc as bacc
import concourse.tile as tile
import concourse.mybir as mybir
from concourse.bass_utils import run_bass_kernel_spmd

# problem dims (fixed by the graded problem)
T, D, I, E = 1024, 2048, 1408, 16
SI = 2 * I               # shared expert intermediate (2816)
TOP_K, N_GROUP, TOPK_GROUP = 6, 4, 2
ROUTED_SCALE = 2.5
NCORES = 8
KT = D // 128            # 16 contraction tiles
IH = I // 2              # 704 intermediate rows per half
IHP = 768                # padded to 6x128
HIT = IHP // 128         # 6 intermediate tiles per half
SSLICE = SI // NCORES    # 352 shared-intermediate rows per core
SIP = 384                # padded to 3x128
SIT = SIP // 128         # 3
NT2 = D // 512           # 4 gemm2 n-tiles
NSLOT = 4                # expert slots per core

f32 = mybir.dt.float32
bf16 = mybir.dt.bfloat16
np_bf16 = ml_dtypes.bfloat16
ACT_SILU = mybir.ActivationFunctionType.Silu
ACT_SIGMOID = mybir.ActivationFunctionType.Sigmoid
ACT_COPY = mybir.ActivationFunctionType.Copy
_SIM_SILU = False  # CoreSim lacks Silu; True emits Sigmoid + explicit mul


# ---------------------------------------------------------------- routing
def _route(x, gate_w, bias):
    """Replicates the jax reference gate in numpy f32 (decision margins are
    >=1e-4 so ulp-level differences cannot flip the top-k).

    Returns topk_idx [T,6] int, weights [T,6] f32 (renormalized, unscaled).
    """
    logits = (x @ gate_w.T).astype(np.float32)
    scores = (1.0 / (1.0 + np.exp(-logits))).astype(np.float32)
    s_choice = scores + bias.astype(np.float32)
    grp = s_choice.reshape(T, N_GROUP, E // N_GROUP)
    group_scores = np.sort(grp, axis=2)[:, :, -2:].sum(2, dtype=np.float32)
    grp_idx = np.argsort(-group_scores, axis=1, kind="stable")[:, :TOPK_GROUP]
    gmask = np.zeros((T, N_GROUP), dtype=bool)
    gmask[np.arange(T)[:, None], grp_idx] = True
    emask = np.repeat(gmask, E // N_GROUP, axis=1)
    masked = np.where(emask, s_choice, -np.inf)
    topk_idx = np.argsort(-masked, axis=1, kind="stable")[:, :TOP_K]
    w = np.take_along_axis(scores, topk_idx, axis=1)
    w = (w / w.sum(axis=1, keepdims=True)).astype(np.float32)
    return topk_idx, w


def _chunks(c):
    """Split capacity c into GEMM1 moving-dim chunks, each <= 512."""
    if c <= 512:
        return [(0, c)]
    a = 16 * ((c + 31) // 32)
    return [(0, a), (a, c - a)]


def _pad16(n):
    return max(128, 16 * ((n + 15) // 16))


# ------------------------------------------------------------ host packing
def _pack_wgu(w, it_cnt):
    """w: [2*ic, D] rows (gate block then up block, ic=128*it_cnt rows each)
    -> [2*it_cnt, 128, KT, 128] bf16 with gate/up 128-row tiles interleaved,
    laid out so tile m is a [128 part, KT*128] contiguous block of
    w^T[k-tile, m-tile]."""
    ic = 128 * it_cnt
    g = w[:ic].reshape(it_cnt, 128, D)
    u = w[ic:].reshape(it_cnt, 128, D)
    inter = np.stack([g, u], axis=1).reshape(2 * it_cnt * 128, D)  # [2ic, D]
    t = inter.T.reshape(KT, 128, 2 * it_cnt, 128).transpose(2, 1, 0, 3)
    return np.ascontiguousarray(t).astype(np_bf16)


def _pack_wd(wdT, it_cnt):
    """wdT: [128*it_cnt, D] (= w_down^T, zero-padded rows allowed)
    -> [NT2, 128, it_cnt, 512] bf16."""
    t = wdT.reshape(it_cnt, 128, NT2, 512).transpose(2, 1, 0, 3)
    return np.ascontiguousarray(t).astype(np_bf16)


def _pack_xT(xs, cap):
    """xs: [n, D] token rows -> [128, KT, cap] bf16 (x^T k-tiles, padded)."""
    out = np.zeros((128, KT, cap), dtype=np_bf16)
    n = xs.shape[0]
    out[:, :, :n] = xs.T.reshape(KT, 128, n).transpose(1, 0, 2).astype(np_bf16)
    return out


# ------------------------------------------------------------ device build
def _build(caps):
    nc = bacc.Bacc("TRN2", target_bir_lowering=False, debug=False,
                   num_devices=NCORES)
    slot_mtl = [[(r, min(128, c - r)) for r in range(0, c, 128)] for c in caps]
    mts = [(r, 128) for r in range(0, T, 128)]

    xg_d = [nc.dram_tensor(f"xg{s}", [128, KT, caps[s]], bf16,
                           kind="ExternalInput") for s in range(NSLOT)]
    wgu_d = [nc.dram_tensor(f"wgu{s}", [2 * HIT, 128, KT, 128], bf16,
                            kind="ExternalInput") for s in range(NSLOT)]
    wd_d = [nc.dram_tensor(f"wd{s}", [NT2, 128, HIT, 512], bf16,
                           kind="ExternalInput") for s in range(NSLOT)]
    cw_d = [nc.dram_tensor(f"cw{s}", [128, len(slot_mtl[s])], f32,
                           kind="ExternalInput") for s in range(NSLOT)]
    yr_d = [nc.dram_tensor(f"yr{s}", [caps[s], D], bf16,
                           kind="ExternalOutput") for s in range(NSLOT)]
    xt_d = nc.dram_tensor("xt", [128, KT, T], bf16, kind="ExternalInput")
    wsgu_d = nc.dram_tensor("wsgu", [2 * SIT, 128, KT, 128], bf16, kind="ExternalInput")
    wsd_d = nc.dram_tensor("wsd", [NT2, 128, SIT, 512], bf16, kind="ExternalInput")
    ys_d = nc.dram_tensor("ys", [T, D], bf16, kind="ExternalOutput")

    with tile.TileContext(nc) as tc, ExitStack() as ctx:
        sb = ctx.enter_context(tc.tile_pool(name="sb", bufs=1))
        ps = ctx.enter_context(tc.tile_pool(name="ps", bufs=1, space="PSUM"))

        def gemm1(xgd, cap, chunks, it_cnt, wgud, cwd, n_mt):
            """Emit x load + GEMM1 + silu*mul; returns (at, cw) tiles."""
            if cap == T:
                # shared-expert x^T: own tags so its DMA prefetches early
                h = KT // 2
                xa = sb.tile([128, h, cap], bf16, tag="xta", bufs=1, name="xa")
                xb = sb.tile([128, KT - h, cap], bf16, tag="xtb", bufs=1, name="xb")
                nc.gpsimd.dma_start(xa[:], xgd.ap()[:, :h, :])
                nc.gpsimd.dma_start(xb[:], xgd.ap()[:, h:, :])
                xg_at = lambda k: xa[:, k, :] if k < h else xb[:, k - h, :]
            else:
                xg = sb.tile([128, KT, cap], bf16, tag="xbuf", bufs=2, name="xg")
                h = KT // 2
                nc.gpsimd.dma_start(xg[:, :h, :], xgd.ap()[:, :h, :])
                nc.gpsimd.dma_start(xg[:, h:, :], xgd.ap()[:, h:, :])
                xg_at = lambda k: xg[:, k, :]
            cw = None
            if cwd is not None:
                cw = sb.tile([128, n_mt], f32, tag="cw", bufs=2, name="cw")
                nc.gpsimd.dma_start(cw[:], cwd.ap())
            at = sb.tile([128, it_cnt, cap], bf16, tag="at", bufs=3, name="at")
            for t in range(it_cnt):
                pair = []
                for par in (0, 1):
                    wgu = sb.tile([128, KT, 128], bf16, tag="wgu", bufs=6, name="wgu")
                    nc.sync.dma_start(wgu[:], wgud.ap()[2 * t + par])
                    row = []
                    for off, n in chunks:
                        p = ps.tile([128, n], f32, tag=f"ps{par}", bufs=3, name=f"ps{par}")
                        for k in range(KT):
                            nc.tensor.matmul(p[:], wgu[:, k, :], xg_at(k)[:, off:off + n],
                                             start=(k == 0), stop=(k == KT - 1))
                        row.append(p)
                    pair.append(row)
                for ci, (off, n) in enumerate(chunks):
                    tmp = sb.tile([128, n], bf16, tag="tmp", bufs=3, name="tmp")
                    if _SIM_SILU:
                        nc.scalar.activation(tmp[:], pair[0][ci][:], ACT_SIGMOID)
                        nc.vector.tensor_mul(tmp[:], tmp[:], pair[0][ci][:])
                    else:
                        nc.scalar.activation(tmp[:], pair[0][ci][:], ACT_SILU)
                    nc.vector.tensor_mul(at[:, t, off:off + n], tmp[:], pair[1][ci][:])
            return at, cw

        def gemm2(at, cw, it_cnt, wdd, mtl, out_d):
            """Emit GEMM2 + per-token scale + output DMA."""
            for nt in range(NT2):
                wd = sb.tile([128, it_cnt, 512], bf16, tag="wd", bufs=3, name="wd")
                nc.sync.dma_start(wd[:], wdd.ap()[nt])
                for mi, (r0, p_) in enumerate(mtl):
                    yp = ps.tile([128, 512], f32, tag="psy", bufs=2, name="yp")
                    for k in range(it_cnt):
                        nc.tensor.matmul(yp[:p_, :], at[:, k, r0:r0 + p_], wd[:, k, :],
                                         start=(k == 0), stop=(k == it_cnt - 1))
                    ysb = sb.tile([128, 512], bf16, tag="ysb", bufs=4, name="ysb")
                    if cw is not None:
                        nc.scalar.activation(ysb[:p_, :], yp[:p_, :], ACT_COPY,
                                             scale=cw[:p_, mi:mi + 1])
                    else:
                        nc.scalar.activation(ysb[:p_, :], yp[:p_, :], ACT_COPY)
                    nc.scalar.dma_start(out_d.ap()[r0:r0 + p_, nt * 512:(nt + 1) * 512],
                                        ysb[:p_, :])

        # slots smallest-first (short startup x-DMA chase); shared expert
        # last.  GEMM1 of phase i+1 is emitted before GEMM2 of phase i so
        # the PE stream never waits on a slot's silu/mul tail.
        phases = []
        for s in reversed(range(NSLOT)):
            phases.append(dict(xgd=xg_d[s], cap=caps[s], chunks=_chunks(caps[s]),
                               it=HIT, wgud=wgu_d[s], cwd=cw_d[s],
                               wdd=wd_d[s], mtl=slot_mtl[s], out=yr_d[s]))
        phases.append(dict(xgd=xt_d, cap=T, chunks=[(0, 512), (512, 512)],
                           it=SIT, wgud=wsgu_d, cwd=None,
                           wdd=wsd_d, mtl=mts, out=ys_d))
        pend = None
        for ph in phases:
            a = gemm1(ph["xgd"], ph["cap"], ph["chunks"], ph["it"],
                      ph["wgud"], ph["cwd"], len(ph["mtl"]))
            if pend is not None:
                gemm2(pend[0], pend[1], pend[2]["it"], pend[2]["wdd"],
                      pend[2]["mtl"], pend[2]["out"])
            pend = (a[0], a[1], ph)
        gemm2(pend[0], pend[1], pend[2]["it"], pend[2]["wdd"],
              pend[2]["mtl"], pend[2]["out"])

    nc.compile()
    return nc


# ----------------------------------------------------------------- kernel
def kernel(x, gate_w, bias, w_gate_up, w_down, shared_w_gate_up,
           shared_w_down, _trace=False):
    x = np.ascontiguousarray(x, dtype=np.float32)
    topk_idx, w = _route(x, gate_w, bias)
    cw_full = w.astype(np.float32) * np.float32(ROUTED_SCALE)

    # expert -> token list + weight list
    toks, wts, counts = [], [], np.zeros(E, dtype=np.int64)
    for e in range(E):
        tsel, ksel = np.where(topk_idx == e)
        toks.append(tsel)
        wts.append(cw_full[tsel, ksel])
        counts[e] = len(tsel)

    # rank-sorted experts dealt into 4 slots x 4 groups; group p -> cores
    # (2p, 2p+1), each core computing one half of the intermediate dim.
    order = np.argsort(-counts, kind="stable")
    slot_experts = [[int(order[4 * s + p]) for p in range(4)] for s in range(NSLOT)]
    caps = [_pad16(int(max(counts[e] for e in slot_experts[s])))
            for s in range(NSLOT)]
    n_mt = [(caps[s] + 127) // 128 for s in range(NSLOT)]

    # pack per (group, slot, half) once; xg/cw shared by both cores of a pair
    xt_pack = _pack_xT(x, T)
    in_maps = []
    for c in range(NCORES):
        p, h = c // 2, c % 2
        m = {}
        for s in range(NSLOT):
            eid = slot_experts[s][p]
            if h == 0:
                m[f"xg{s}"] = _pack_xT(x[toks[eid]], caps[s])
                cwv = np.zeros(n_mt[s] * 128, dtype=np.float32)
                cwv[:counts[eid]] = wts[eid]
                m[f"cw{s}"] = np.ascontiguousarray(cwv.reshape(n_mt[s], 128).T)
            else:
                m[f"xg{s}"] = in_maps[c - 1][f"xg{s}"]
                m[f"cw{s}"] = in_maps[c - 1][f"cw{s}"]
            # half h of the expert's intermediate rows, zero-padded 704->768
            gsl = np.zeros((2 * IHP, D), dtype=np.float32)
            gsl[:IH] = w_gate_up[eid][IH * h: IH * (h + 1)]
            gsl[IHP:IHP + IH] = w_gate_up[eid][I + IH * h: I + IH * (h + 1)]
            m[f"wgu{s}"] = _pack_wgu(gsl, HIT)
            sdT = np.zeros((IHP, D), dtype=np.float32)
            sdT[:IH] = w_down[eid].T[IH * h: IH * (h + 1)]
            m[f"wd{s}"] = _pack_wd(sdT, HIT)
        # shared expert slice (rows [352c, 352c+352), zero-padded to 384)
        gsl = np.zeros((2 * SIP, D), dtype=np.float32)
        gsl[:SSLICE] = shared_w_gate_up[SSLICE * c: SSLICE * (c + 1)]
        gsl[SIP:SIP + SSLICE] = shared_w_gate_up[SI + SSLICE * c: SI + SSLICE * (c + 1)]
        m["wsgu"] = _pack_wgu(gsl, SIT)
        sdT = np.zeros((SIP, D), dtype=np.float32)
        sdT[:SSLICE] = shared_w_down[:, SSLICE * c: SSLICE * (c + 1)].T
        m["wsd"] = _pack_wd(sdT, SIT)
        m["xt"] = xt_pack
        in_maps.append(m)

    nc = _build(caps)
    kw = {}
    if _trace:
        kw = dict(trace=True, trace_cores=list(range(NCORES)))
    res = run_bass_kernel_spmd(nc, in_maps, core_ids=list(range(NCORES)), **kw)

    y = np.zeros((T, D), dtype=np.float32)
    for c in range(NCORES):
        y += res.results[c]["ys"].astype(np.float32)
    for c in range(NCORES):
        p = c // 2
        for s in range(NSLOT):
            eid = slot_experts[s][p]
            n = int(counts[eid])
            if n:
                y[toks[eid]] += res.results[c][f"yr{s}"][:n].astype(np.float32)
    if _trace:
        return y, res
    return y


# revision 8
# speedup vs baseline: 1.1125x; 1.1125x over previous
"""DeepseekV2 MoE layer on 8 Trainium2 NeuronCores (Bass/Tile, SPMD).

Strategy (expert-parallel with intermediate-dim pair-split, bf16 matmuls):
 - Host computes the MoE gate routing in numpy (bitwise-matches the jax
   reference: top-k margins are ~1e-4, far above ulp noise).
 - 16 experts, rank-sorted by token count, are dealt into 4 groups of 4
   (group p = ranks {p, 4+p, 8+p, 12+p}).  Group p is owned by the core
   pair (2p, 2p+1): both cores process the SAME gathered tokens of all 4
   experts, but each core only computes HALF of every expert's
   intermediate dim (704 rows, zero-padded to 768 = 6 tile-pairs).  The
   two half outputs are partial sums; the host adds them.  This beats
   whole-expert placement because per-slot capacity is the max count at
   ranks {0,4,8,12} (688+432+352+272) instead of ranks {0,8} (688+352)
   at double width.
 - Device per core: for each of 4 slots, GEMM1 (x_gathered^T @ half
   w_gate_up^T, gate/up rows interleaved in 128-row pairs) -> SiLU*mul
   -> GEMM2 (half w_down) -> scale rows by renormalized gate weight *
   2.5 -> DMA out.  Then the shared expert, TP-sharded over its
   intermediate dim (352 per core, zero-padded to 384).
 - Phases are emitted interleaved (GEMM1 of slot s+1 before GEMM2 of
   slot s) so the PE never idles on a slot's silu/mul tail; slots run
   smallest-first so the startup x-DMA chase is short.
 - All matmuls run in bf16 (fp32 PSUM accumulate): halves HBM traffic
   and enables fast-weight-load; rel err ~5e-3 vs the 2e-2 gate.
 - Host scatter-adds per-expert partial outputs and shared partials, f32.
"""

import numpy as np
import ml_dtypes
from contextlib import ExitStack

import concourse.bacc as bacc
import concourse.tile as tile
import concourse.mybir as mybir
from concourse.bass_utils import run_bass_kernel_spmd

# problem dims (fixed by the graded problem)
T, D, I, E = 1024, 2048, 1408, 16
SI = 2 * I               # shared expert intermediate (2816)
TOP_K, N_GROUP, TOPK_GROUP = 6, 4, 2
ROUTED_SCALE = 2.5
NCORES = 8
KT = D // 128            # 16 contraction tiles
IH = I // 2              # 704 intermediate rows per half
IHP = 768                # padded to 6x128
HIT = IHP // 128         # 6 intermediate tiles per half
SSLICE = SI // NCORES    # 352 shared-intermediate rows per core
SIP = 384                # padded to 3x128
SIT = SIP // 128         # 3
NT2 = D // 512           # 4 gemm2 n-tiles
NSLOT = 4                # expert slots per core

f32 = mybir.dt.float32
bf16 = mybir.dt.bfloat16
np_bf16 = ml_dtypes.bfloat16
ACT_SILU = mybir.ActivationFunctionType.Silu
ACT_SIGMOID = mybir.ActivationFunctionType.Sigmoid
ACT_COPY = mybir.ActivationFunctionType.Copy
_SIM_SILU = False  # CoreSim lacks Silu; True emits Sigmoid + explicit mul


# ---------------------------------------------------------------- routing
def _route(x, gate_w, bias):
    """Replicates the jax reference gate in numpy f32 (decision margins are
    >=1e-4 so ulp-level differences cannot flip the top-k).

    Returns topk_idx [T,6] int, weights [T,6] f32 (renormalized, unscaled).
    """
    logits = (x @ gate_w.T).astype(np.float32)
    scores = (1.0 / (1.0 + np.exp(-logits))).astype(np.float32)
    s_choice = scores + bias.astype(np.float32)
    grp = s_choice.reshape(T, N_GROUP, E // N_GROUP)
    group_scores = np.sort(grp, axis=2)[:, :, -2:].sum(2, dtype=np.float32)
    grp_idx = np.argsort(-group_scores, axis=1, kind="stable")[:, :TOPK_GROUP]
    gmask = np.zeros((T, N_GROUP), dtype=bool)
    gmask[np.arange(T)[:, None], grp_idx] = True
    emask = np.repeat(gmask, E // N_GROUP, axis=1)
    masked = np.where(emask, s_choice, -np.inf)
    topk_idx = np.argsort(-masked, axis=1, kind="stable")[:, :TOP_K]
    w = np.take_along_axis(scores, topk_idx, axis=1)
    w = (w / w.sum(axis=1, keepdims=True)).astype(np.float32)
    return topk_idx, w


def _chunks(c):
    """Split capacity c into GEMM1 moving-dim chunks, each <= 512."""
    if c <= 512:
        return [(0, c)]
    a = 16 * ((c + 31) // 32)
    return [(0, a), (a, c - a)]


def _pad16(n):
    return max(128, 16 * ((n + 15) // 16))


# ------------------------------------------------------------ host packing
def _pack_wgu(w, it_cnt):
    """w: [2*ic, D] rows (gate block then up block, ic=128*it_cnt rows each)
    -> [2*it_cnt, 128, KT, 128] bf16 with gate/up 128-row tiles interleaved,
    laid out so tile m is a [128 part, KT*128] contiguous block of
    w^T[k-tile, m-tile]."""
    ic = 128 * it_cnt
    g = w[:ic].reshape(it_cnt, 128, D)
    u = w[ic:].reshape(it_cnt, 128, D)
    inter = np.stack([g, u], axis=1).reshape(2 * it_cnt * 128, D)  # [2ic, D]
    t = inter.T.reshape(KT, 128, 2 * it_cnt, 128).transpose(2, 1, 0, 3)
    return np.ascontiguousarray(t).astype(np_bf16)


def _pack_wd(wdT, it_cnt):
    """wdT: [128*it_cnt, D] (= w_down^T, zero-padded rows allowed)
    -> [NT2, 128, it_cnt, 512] bf16."""
    t = wdT.reshape(it_cnt, 128, NT2, 512).transpose(2, 1, 0, 3)
    return np.ascontiguousarray(t).astype(np_bf16)


def _pack_xT(xs, cap):
    """xs: [n, D] token rows -> [128, KT, cap] bf16 (x^T k-tiles, padded)."""
    out = np.zeros((128, KT, cap), dtype=np_bf16)
    n = xs.shape[0]
    out[:, :, :n] = xs.T.reshape(KT, 128, n).transpose(1, 0, 2).astype(np_bf16)
    return out


# ------------------------------------------------------------ device build
def _build(caps):
    nc = bacc.Bacc("TRN2", target_bir_lowering=False, debug=False,
                   num_devices=NCORES)
    slot_mtl = [[(r, min(128, c - r)) for r in range(0, c, 128)] for c in caps]
    mts = [(r, 128) for r in range(0, T, 128)]

    n_mt_tot = sum(len(m) for m in slot_mtl)
    xg_d = [nc.dram_tensor(f"xg{s}", [128, KT, caps[s]], bf16,
                           kind="ExternalInput") for s in range(NSLOT)]
    wgu_d = [nc.dram_tensor(f"wgu{s}", [2 * HIT, 128, KT, 128], bf16,
                            kind="ExternalInput") for s in range(NSLOT)]
    wd_d = [nc.dram_tensor(f"wd{s}", [NT2, 128, HIT, 512], bf16,
                           kind="ExternalInput") for s in range(NSLOT)]
    cw_d = nc.dram_tensor("cw", [128, n_mt_tot], f32, kind="ExternalInput")
    yr_d = [nc.dram_tensor(f"yr{s}", [caps[s], D], bf16,
                           kind="ExternalOutput") for s in range(NSLOT)]
    xt_d = nc.dram_tensor("xt", [128, KT, T], bf16, kind="ExternalInput")
    wsgu_d = nc.dram_tensor("wsgu", [2 * SIT, 128, KT, 128], bf16, kind="ExternalInput")
    wsd_d = nc.dram_tensor("wsd", [NT2, 128, SIT, 512], bf16, kind="ExternalInput")
    ys_d = nc.dram_tensor("ys", [T, D], bf16, kind="ExternalOutput")

    with tile.TileContext(nc) as tc, ExitStack() as ctx:
        sb = ctx.enter_context(tc.tile_pool(name="sb", bufs=1))
        ps = ctx.enter_context(tc.tile_pool(name="ps", bufs=1, space="PSUM"))

        def gemm1(xgd, cap, chunks, it_cnt, wgud, xtag):
            """Emit x load + GEMM1 + silu*mul; returns the at tile."""
            # per-phase x tag (bufs=1, exclusive) so every phase's x load can
            # start at kernel t=0; 4 k-block sub-DMAs let matmuls chase the
            # transfer at k-granularity.
            xg = sb.tile([128, KT, cap], bf16, tag=xtag, bufs=1, name=xtag)
            for kb in range(0, KT, 4):
                nc.gpsimd.dma_start(xg[:, kb:kb + 4, :], xgd.ap()[:, kb:kb + 4, :])
            xg_at = lambda k: xg[:, k, :]
            at = sb.tile([128, it_cnt, cap], bf16, tag="at", bufs=3, name="at")
            for t in range(it_cnt):
                pair = []
                for par in (0, 1):
                    wgu = sb.tile([128, KT, 128], bf16, tag="wgu", bufs=6, name="wgu")
                    nc.sync.dma_start(wgu[:], wgud.ap()[2 * t + par])
                    row = []
                    for off, n in chunks:
                        p = ps.tile([128, n], f32, tag=f"ps{par}", bufs=3, name=f"ps{par}")
                        for k in range(KT):
                            nc.tensor.matmul(p[:], wgu[:, k, :], xg_at(k)[:, off:off + n],
                                             start=(k == 0), stop=(k == KT - 1))
                        row.append(p)
                    pair.append(row)
                for ci, (off, n) in enumerate(chunks):
                    tmp = sb.tile([128, n], bf16, tag="tmp", bufs=3, name="tmp")
                    if _SIM_SILU:
                        nc.scalar.activation(tmp[:], pair[0][ci][:], ACT_SIGMOID)
                        nc.vector.tensor_mul(tmp[:], tmp[:], pair[0][ci][:])
                    else:
                        nc.scalar.activation(tmp[:], pair[0][ci][:], ACT_SILU)
                    nc.vector.tensor_mul(at[:, t, off:off + n], tmp[:], pair[1][ci][:])
            return at

        cw = sb.tile([128, n_mt_tot], f32, tag="cw", bufs=1, name="cw")
        nc.gpsimd.dma_start(cw[:], cw_d.ap())

        def gemm2(at, cw_off, it_cnt, wdd, mtl, out_d):
            """Emit GEMM2 + per-token scale + output DMA."""
            for nt in range(NT2):
                wd = sb.tile([128, it_cnt, 512], bf16, tag="wd", bufs=4, name="wd")
                nc.sync.dma_start(wd[:], wdd.ap()[nt])
                for mi, (r0, p_) in enumerate(mtl):
                    yp = ps.tile([128, 512], f32, tag="psy", bufs=2, name="yp")
                    for k in range(it_cnt):
                        nc.tensor.matmul(yp[:p_, :], at[:, k, r0:r0 + p_], wd[:, k, :],
                                         start=(k == 0), stop=(k == it_cnt - 1))
                    ysb = sb.tile([128, 512], bf16, tag="ysb", bufs=4, name="ysb")
                    if cw_off is not None:
                        nc.vector.tensor_scalar_mul(
                            ysb[:p_, :], yp[:p_, :],
                            cw[:p_, cw_off + mi:cw_off + mi + 1])
                    else:
                        nc.vector.tensor_copy(ysb[:p_, :], yp[:p_, :])
                    nc.scalar.dma_start(out_d.ap()[r0:r0 + p_, nt * 512:(nt + 1) * 512],
                                        ysb[:p_, :])

        # slots largest-first (weight-DMA-hungry small slots get prefetch
        # slack); shared expert last.  GEMM1 of phase i+1 is emitted before
        # GEMM2 of phase i so the PE stream never waits on a silu/mul tail.
        phases = []
        cw_off = 0
        for s in range(NSLOT):
            phases.append(dict(xgd=xg_d[s], cap=caps[s], chunks=_chunks(caps[s]),
                               it=HIT, wgud=wgu_d[s], cwo=cw_off,
                               wdd=wd_d[s], mtl=slot_mtl[s], out=yr_d[s],
                               xtag=f"xb{s}"))
            cw_off += len(slot_mtl[s])
        phases.append(dict(xgd=xt_d, cap=T, chunks=[(0, 512), (512, 512)],
                           it=SIT, wgud=wsgu_d, cwo=None,
                           wdd=wsd_d, mtl=mts, out=ys_d, xtag="xts"))
        pend = None
        for ph in phases:
            a = gemm1(ph["xgd"], ph["cap"], ph["chunks"], ph["it"],
                      ph["wgud"], ph["xtag"])
            if pend is not None:
                gemm2(pend[0], pend[1]["cwo"], pend[1]["it"], pend[1]["wdd"],
                      pend[1]["mtl"], pend[1]["out"])
            pend = (a, ph)
        gemm2(pend[0], pend[1]["cwo"], pend[1]["it"], pend[1]["wdd"],
              pend[1]["mtl"], pend[1]["out"])

    nc.compile()
    return nc


# ----------------------------------------------------------------- kernel
def kernel(x, gate_w, bias, w_gate_up, w_down, shared_w_gate_up,
           shared_w_down, _trace=False):
    x = np.ascontiguousarray(x, dtype=np.float32)
    topk_idx, w = _route(x, gate_w, bias)
    cw_full = w.astype(np.float32) * np.float32(ROUTED_SCALE)

    # expert -> token list + weight list
    toks, wts, counts = [], [], np.zeros(E, dtype=np.int64)
    for e in range(E):
        tsel, ksel = np.where(topk_idx == e)
        toks.append(tsel)
        wts.append(cw_full[tsel, ksel])
        counts[e] = len(tsel)

    # rank-sorted experts dealt into 4 slots x 4 groups; group p -> cores
    # (2p, 2p+1), each core computing one half of the intermediate dim.
    order = np.argsort(-counts, kind="stable")
    slot_experts = [[int(order[4 * s + p]) for p in range(4)] for s in range(NSLOT)]
    caps = [_pad16(int(max(counts[e] for e in slot_experts[s])))
            for s in range(NSLOT)]
    n_mt = [(caps[s] + 127) // 128 for s in range(NSLOT)]

    # pack per (group, slot, half) once; xg/cw shared by both cores of a pair
    xt_pack = _pack_xT(x, T)
    in_maps = []
    for c in range(NCORES):
        p, h = c // 2, c % 2
        m = {}
        if h == 0:
            cwcols = []
            for s in range(NSLOT):
                eid = slot_experts[s][p]
                cwv = np.zeros(n_mt[s] * 128, dtype=np.float32)
                cwv[:counts[eid]] = wts[eid]
                cwcols.append(cwv.reshape(n_mt[s], 128).T)
            m["cw"] = np.ascontiguousarray(np.concatenate(cwcols, axis=1))
        else:
            m["cw"] = in_maps[c - 1]["cw"]
        for s in range(NSLOT):
            eid = slot_experts[s][p]
            if h == 0:
                m[f"xg{s}"] = _pack_xT(x[toks[eid]], caps[s])
            else:
                m[f"xg{s}"] = in_maps[c - 1][f"xg{s}"]
            # half h of the expert's intermediate rows, zero-padded 704->768
            gsl = np.zeros((2 * IHP, D), dtype=np.float32)
            gsl[:IH] = w_gate_up[eid][IH * h: IH * (h + 1)]
            gsl[IHP:IHP + IH] = w_gate_up[eid][I + IH * h: I + IH * (h + 1)]
            m[f"wgu{s}"] = _pack_wgu(gsl, HIT)
            sdT = np.zeros((IHP, D), dtype=np.float32)
            sdT[:IH] = w_down[eid].T[IH * h: IH * (h + 1)]
            m[f"wd{s}"] = _pack_wd(sdT, HIT)
        # shared expert slice (rows [352c, 352c+352), zero-padded to 384)
        gsl = np.zeros((2 * SIP, D), dtype=np.float32)
        gsl[:SSLICE] = shared_w_gate_up[SSLICE * c: SSLICE * (c + 1)]
        gsl[SIP:SIP + SSLICE] = shared_w_gate_up[SI + SSLICE * c: SI + SSLICE * (c + 1)]
        m["wsgu"] = _pack_wgu(gsl, SIT)
        sdT = np.zeros((SIP, D), dtype=np.float32)
        sdT[:SSLICE] = shared_w_down[:, SSLICE * c: SSLICE * (c + 1)].T
        m["wsd"] = _pack_wd(sdT, SIT)
        m["xt"] = xt_pack
        in_maps.append(m)

    nc = _build(caps)
    kw = {}
    if _trace:
        kw = dict(trace=True, trace_cores=list(range(NCORES)))
    res = run_bass_kernel_spmd(nc, in_maps, core_ids=list(range(NCORES)), **kw)

    y = np.zeros((T, D), dtype=np.float32)
    for c in range(NCORES):
        y += res.results[c]["ys"].astype(np.float32)
    for c in range(NCORES):
        p = c // 2
        for s in range(NSLOT):
            eid = slot_experts[s][p]
            n = int(counts[eid])
            if n:
                y[toks[eid]] += res.results[c][f"yr{s}"][:n].astype(np.float32)
    if _trace:
        return y, res
    return y
